# revision 27
# baseline (speedup 1.0000x reference)
"""Trainium2 Bass kernel for nn_EventSplitter (edge-restricted graph transformer).

kernel(**inputs) takes the FULL unsharded numpy inputs and returns the FULL
[E, 1] float32 output.

Sharding / schedule (8 NeuronCores, one SPMD program):
  - Nodes padded to 8*B*128; core c owns nodes [c*B*128, (c+1)*B*128).
  - Edges assigned to the core owning their dst. Per dst-block (128 nodes),
    edges are split into PHASE-1 chunks (K1 per block; only edges whose src
    lives on this core's PAIR, exploiting pair-shared DRAM scratchpad) and
    PHASE-2 chunks (the rest). Phase-1 gathers run DURING the kv AllGather
    (they read a pair-shared table filled by direct scatter + a tiny
    barrier AllGather); phase-2 gathers read the collective's output.
  - S (edge->node one-hot) and its transpose are computed once at setup and
    streamed from DRAM each layer.
  - Per-chunk: one indirect gather-add of [k|v] rows onto an e-projection
    initialized buffer; DVE ops batched 2 chunks wide; segment softmax via
    PSUM-accumulated S matmuls as in the reference.
"""

import math

import numpy as np

import concourse.bass as bass
import concourse.tile as tile
import concourse.mybir as mybir
from concourse.masks import make_identity

from contextlib import ExitStack

# --- walrus sync-command workaround (inlined; see waitsplit.py) ---

_ctr = [0]

_ZERO_SYNC_TYPES = ("InstIota",)


def _mk_nop(engine, waits, updates):
    nop = mybir.InstNoOp(name=f"wsplit-{_ctr[0]}", ins=[], outs=[])
    _ctr[0] += 1
    nop.engine = engine
    nop.sync_info = mybir.SyncInfo(on_wait=list(waits), on_update=list(updates))
    return nop


def split_excess_waits(nc, max_waits=1):
    for f in nc.m.functions:
        for bb in f.blocks:
            out = []
            changed = False
            for ins in bb.instructions:
                si = ins.sync_info
                zero_sync = type(ins).__name__ in _ZERO_SYNC_TYPES
                if si is None:
                    out.append(ins)
                    continue
                waits = list(si.on_wait)
                updates = list(si.on_update)
                limit = 0 if zero_sync else max_waits
                post_updates = updates if zero_sync and updates else []
                if len(waits) > limit or post_updates:
                    keep_w = waits[len(waits) - limit:] if limit else []
                    extra_w = waits[:len(waits) - limit] if limit else waits
                    step = max(1, max_waits)
                    for i in range(0, len(extra_w), step):
                        out.append(_mk_nop(ins.engine, extra_w[i:i + step], []))
                    ins.sync_info = mybir.SyncInfo(
                        on_wait=list(keep_w),
                        on_update=[] if post_updates else list(updates))
                    out.append(ins)
                    if post_updates:
                        out.append(_mk_nop(ins.engine, [], post_updates))
                    changed = True
                else:
                    out.append(ins)
            if changed:
                bb.instructions[:] = out
    return nc


P = 128
dt = mybir.dt

HID = 192
HEADS = 4
DH = 48
L = 3
FFN = 384
EA = 4
GP = 3
SP_ = 3
EP = 6
G_TBL = 512
XS = 7          # x(4) + splitter_probs(3)
H2 = 2 * HID    # 384
Z2 = 96
K1 = 2          # phase-1 (pair-local) chunks per dst block
_DISABLE_PHASE1 = [False]
_DEBUG = [False]
dbg_ctx = [0]


# ----------------------------------------------------------------------------
# host-side sharding / index prep
# ----------------------------------------------------------------------------

def _host_prep(x, edge_index, edge_attr, batch, group_ptr, time_group_ids,
               group_probs, splitter_probs, endpoint_preds, n_cores):
    N = x.shape[0]
    E = edge_index.shape[1]
    B = int(math.ceil(N / (n_cores * P)))     # dst-blocks per core
    NLp = B * P                               # local nodes per core (padded)
    Np = NLp * n_cores

    src = np.asarray(edge_index[0], np.int64)
    dst = np.asarray(edge_index[1], np.int64)

    gids = np.clip(np.asarray(group_ptr)[np.asarray(batch)] + np.asarray(time_group_ids),
                   0, group_probs.shape[0] - 1).astype(np.int64)

    n_blocks_total = Np // P
    blk_of_edge = dst // P
    order = np.argsort(dst, kind="stable")
    cnt = np.bincount(blk_of_edge, minlength=n_blocks_total)

    sorted_eids = order
    sorted_blk = blk_of_edge[order]
    startpos = np.zeros(n_blocks_total + 1, np.int64)
    np.cumsum(cnt, out=startpos[1:])

    core_of_blk = np.arange(n_blocks_total) // B
    pair_of_core = np.arange(n_cores) // 2
    src_core = src // NLp
    # phase-1 eligible: src on the same PAIR as the dst core
    # split per global block: phase1 edges (<= K1*P), rest
    ph1_cap = K1 * P
    rest_max = 0
    per_blk = []
    for gb in range(n_blocks_total):
        eids = sorted_eids[startpos[gb]:startpos[gb + 1]]
        c = core_of_blk[gb]
        is_pair = pair_of_core[src_core[eids]] == pair_of_core[c]
        if _DISABLE_PHASE1[0]:
            is_pair = np.zeros_like(is_pair)
        p1 = eids[is_pair][:ph1_cap]
        rest = np.concatenate([eids[is_pair][ph1_cap:], eids[~is_pair]])
        per_blk.append((p1, rest))
        rest_max = max(rest_max, len(rest))
    K2 = max(1, int(math.ceil(rest_max / P)))
    K = K1 + K2
    C = B * K
    Ep = C * P

    src_sh = np.zeros((n_cores, Ep), np.int64)     # global src per slot
    amask_sh = np.zeros((n_cores, Ep), np.float32)
    dstc_sh = np.zeros((n_cores, Ep), np.float16)
    ea_sh = np.zeros((n_cores, Ep, EA), np.float16)
    eid_sh = np.full((n_cores, Ep), -1, np.int64)

    ea16 = np.asarray(edge_attr, np.float16)
    for gb in range(n_blocks_total):
        c = core_of_blk[gb]
        bl = gb % B
        p1, rest = per_blk[gb]
        base = bl * K * P
        for eids, off in ((p1, 0), (rest, K1 * P)):
            s = base + off + np.arange(len(eids))
            src_sh[c, s] = src[eids]
            dstc_sh[c, s] = (dst[eids] % P).astype(np.float16)
            amask_sh[c, s] = 1.0
            ea_sh[c, s] = ea16[eids]
            eid_sh[c, s] = eids

    # gather offsets: phase-1 chunks index the PAIR table (2*NLp rows,
    # rows [par*NLp.. ] hold slice of core 2*pair+par); phase-2 index kv_tbl.
    offs_sh = np.zeros((n_cores, Ep), np.int32)
    for c in range(n_cores):
        sl = src_sh[c].reshape(B, K, P)
        of = np.zeros((B, K, P), np.int32)
        # phase-2: global row id
        of[:, K1:, :] = sl[:, K1:, :].astype(np.int32)
        # phase-1: pair-local row id
        pair_base = (c // 2) * 2 * NLp
        of[:, :K1, :] = (sl[:, :K1, :] - pair_base).astype(np.int32)
        # padded slots (amask 0) keep offset 0 which is always valid
        am = amask_sh[c].reshape(B, K, P)
        of[am == 0.0] = 0
        offs_sh[c] = of.reshape(Ep)

    # scatter offsets for own kv slice into pair table
    sco = np.zeros((n_cores, NLp, 1), np.int32)
    for c in range(n_cores):
        sco[c, :, 0] = (c % 2) * NLp + np.arange(NLp)

    xsp = np.concatenate([np.asarray(x, np.float32),
                          np.asarray(splitter_probs, np.float32)], axis=1)
    xsp_p = np.zeros((Np, XS), np.float32)
    xsp_p[:N] = xsp
    gids_p = np.zeros(Np, np.float32)
    gids_p[:N] = gids.astype(np.float32)

    # ea augmented with a ones-row (for edge-head bias folding)
    ea5 = np.concatenate([ea_sh, np.ones((n_cores, Ep, 1), np.float16)], axis=2)

    shards = []
    for c in range(n_cores):
        lo = c * NLp
        shards.append(dict(
            off_src=np.ascontiguousarray(offs_sh[c].reshape(C, P).T),     # [128, C] i32
            dstcol=np.ascontiguousarray(dstc_sh[c].reshape(C, P).T),      # [128, C] f16
            amask=np.ascontiguousarray(amask_sh[c].reshape(C, P).T),      # [128, C] f32
            eaT=np.ascontiguousarray(
                ea5[c].reshape(C, P, EA + 1).transpose(2, 0, 1)
                .reshape(EA + 1, Ep)),                                    # [5, Ep] f16
            xspT=np.ascontiguousarray(xsp_p[lo:lo + NLp].T),              # [7, NLp] f32
            gidcol=np.ascontiguousarray(
                gids_p[lo:lo + NLp].reshape(B, P).T),                     # [128, B] f32
            sco=sco[c],                                                   # [NLp, 1] i32
            barz=np.zeros((64, 1), np.int32),
        ))

    has_pad = (eid_sh < 0).reshape(n_cores, C, P).any(axis=(0, 2))
    meta = dict(N=N, E=E, Np=Np, NLp=NLp, B=B, K=K, C=C, Ep=Ep, eid_sh=eid_sh,
                chunk_pad=[bool(v) for v in has_pad])
    return shards, meta


def _pack_rows(W, dtype):
    """[R, X] -> [128, ceil(R/128), X] partition-chunked, zero padded."""
    W = np.asarray(W, dtype)
    R, X = W.shape
    nck = int(math.ceil(R / P))
    out = np.zeros((P, nck, X), dtype)
    for ci in range(nck):
        r0 = ci * P
        rl = min(P, R - r0)
        out[:rl, ci] = W[r0:r0 + rl]
    return out


def _host_weights(group_probs, endpoint_preds,
                  W_in, b_in, Wq, Wk, Wv, We, Wo, bo, ln1_g, ln1_b,
                  W_ff1, b_ff1, W_ff2, b_ff2, ln2_g, ln2_b,
                  W_e1, b_e1, W_e2, b_e2, W_e3, b_e3):
    f32, f16 = np.float32, np.float16

    def bc(v, X):
        return np.broadcast_to(np.asarray(v, f32)[None, :], (P, X)).copy()

    w = {}
    W_in = np.asarray(W_in, f32)
    w["Wxs"] = np.ascontiguousarray(np.concatenate([W_in[0:4], W_in[7:10]], axis=0))
    w["Wgp"] = np.ascontiguousarray(W_in[4:7])
    w["Wep"] = np.ascontiguousarray(W_in[10:16])
    w["gpT"] = np.ascontiguousarray(np.asarray(group_probs, f32).T)
    w["epT"] = np.ascontiguousarray(np.asarray(endpoint_preds, f32).T)
    w["b_in"] = bc(b_in, HID)

    scale = f32(1.0 / np.sqrt(DH))
    for l in range(L):
        w[f"Wq{l}"] = _pack_rows(np.asarray(Wq[l], f32) * scale, f16)       # [128,2,192]
        w[f"WkWv{l}"] = _pack_rows(np.concatenate(
            [np.asarray(Wk[l], f32), np.asarray(Wv[l], f32)], axis=1), f16)  # [128,2,384]
        w[f"We2_{l}"] = np.concatenate(
            [np.asarray(We[l], f16), np.asarray(We[l], f16)], axis=1)        # [4,384]
        w[f"Wo{l}"] = _pack_rows(Wo[l], f16)
        w[f"bo{l}"] = bc(bo[l], HID)
        w[f"ln1g{l}"] = bc(ln1_g[l], HID)
        w[f"ln1b{l}"] = bc(ln1_b[l], HID)
        w[f"Wff1_{l}"] = _pack_rows(W_ff1[l], f16)                           # [128,2,384]
        w[f"bff1_{l}"] = bc(b_ff1[l], FFN)
        w[f"Wff2_{l}"] = _pack_rows(W_ff2[l], f16)                           # [128,3,192]
        w[f"bff2_{l}"] = bc(b_ff2[l], HID)
        w[f"ln2g{l}"] = bc(ln2_g[l], HID)
        w[f"ln2b{l}"] = bc(ln2_b[l], HID)

    W_e1 = np.asarray(W_e1, f32)
    w["W1a"] = _pack_rows(W_e1[0:HID], f16)
    w["W1b"] = _pack_rows(W_e1[HID:2 * HID], f16)
    # [W1c; b_e1] consumed by the augmented [ea|1] matmul
    w["W1c"] = np.concatenate([np.asarray(W_e1[2 * HID:], f16),
                               np.asarray(b_e1, f16)[None, :]], axis=0)      # [5,192]
    w["W2"] = _pack_rows(W_e2, f16)                                          # [128,2,96]
    w["W3"] = np.asarray(W_e3, f16)                                          # [96,1]
    w["c_iota_row"] = np.broadcast_to(np.arange(P, dtype=f16)[None, :], (P, P)).copy()
    w["ones_row"] = np.ones((1, P), f16)
    for l in range(L):
        w[f"bo_r{l}"] = np.asarray(bo[l], f16)[None, :]
        w[f"bff1_r{l}"] = np.asarray(b_ff1[l], f16)[None, :]
        w[f"bff2_r{l}"] = np.asarray(b_ff2[l], f16)[None, :]
    w["c_iota_col4"] = np.ascontiguousarray(
        (np.arange(G_TBL, dtype=f32).reshape(G_TBL // P, P).T)[:, :, None]
        * np.ones((1, 1, P), f32))
    b_e3c = float(np.asarray(b_e3, f32).reshape(-1)[0])
    return w, b_e3c


# ----------------------------------------------------------------------------
# device program
# ----------------------------------------------------------------------------

def build_program(meta, b_e3_const, n_cores):
    B, K, C, Ep, NLp, Np = (meta["B"], meta["K"], meta["C"], meta["Ep"],
                            meta["NLp"], meta["Np"])
    chunk_pad = meta.get("chunk_pad", [True] * C)
    FC = [(0, P), (P, HID - P)]   # feature chunks of 192
    KP = K * P

    nc = bass.Bass()

    def param(name, shape, dtype):
        return nc.declare_dram_parameter(name, list(shape), dtype, isOutput=False)

    off_src = param("off_src", [P, C], dt.int32)
    dstcol = param("dstcol", [P, C], dt.float16)
    amask = param("amask", [P, C], dt.float32)
    eaT = param("eaT", [EA + 1, Ep], dt.float16)
    xspT = param("xspT", [XS, NLp], dt.float32)
    gidcol = param("gidcol", [P, B], dt.float32)
    sco_p = param("sco", [NLp, 1], dt.int32)
    barz_p = param("barz", [64, 1], dt.int32)
    Wxs = param("Wxs", [XS, HID], dt.float32)
    Wgp = param("Wgp", [GP, HID], dt.float32)
    Wep = param("Wep", [EP, HID], dt.float32)
    gpT = param("gpT", [GP, G_TBL], dt.float32)
    epT = param("epT", [EP, G_TBL], dt.float32)
    b_in = param("b_in", [P, HID], dt.float32)
    Wq_p = [param(f"Wq{l}", [P, 2, HID], dt.float16) for l in range(L)]
    WkWv_p = [param(f"WkWv{l}", [P, 2, H2], dt.float16) for l in range(L)]
    We2_p = [param(f"We2_{l}", [EA, H2], dt.float16) for l in range(L)]
    Wo_p = [param(f"Wo{l}", [P, 2, HID], dt.float16) for l in range(L)]
    bo_p = [param(f"bo{l}", [P, HID], dt.float32) for l in range(L)]
    ln1g_p = [param(f"ln1g{l}", [P, HID], dt.float32) for l in range(L)]
    ln1b_p = [param(f"ln1b{l}", [P, HID], dt.float32) for l in range(L)]
    Wff1_p = [param(f"Wff1_{l}", [P, 2, FFN], dt.float16) for l in range(L)]
    bff1_p = [param(f"bff1_{l}", [P, FFN], dt.float32) for l in range(L)]
    Wff2_p = [param(f"Wff2_{l}", [P, 3, HID], dt.float16) for l in range(L)]
    bff2_p = [param(f"bff2_{l}", [P, HID], dt.float32) for l in range(L)]
    ln2g_p = [param(f"ln2g{l}", [P, HID], dt.float32) for l in range(L)]
    ln2b_p = [param(f"ln2b{l}", [P, HID], dt.float32) for l in range(L)]
    W1a = param("W1a", [P, 2, HID], dt.float16)
    W1b = param("W1b", [P, 2, HID], dt.float16)
    W1c = param("W1c", [EA + 1, HID], dt.float16)
    W2 = param("W2", [P, 2, Z2], dt.float16)
    W3 = param("W3", [Z2, 1], dt.float16)
    c_iota_row = param("c_iota_row", [P, P], dt.float16)
    ones_row_p = param("ones_row", [1, P], dt.float16)
    bo_r_p = [param(f"bo_r{l}", [1, HID], dt.float16) for l in range(L)]
    bff1_r_p = [param(f"bff1_r{l}", [1, FFN], dt.float16) for l in range(L)]
    bff2_r_p = [param(f"bff2_r{l}", [1, HID], dt.float16) for l in range(L)]
    c_iota_col4 = param("c_iota_col4", [P, G_TBL // P, P], dt.float32)

    out_z = nc.declare_dram_parameter("out_z", [1, Ep], dt.float32, isOutput=True)
    debug = _DEBUG[0]
    if debug:
        dbg_h = [nc.declare_dram_parameter(f"dbg_h{i}", [P, B, HID], dt.float32,
                                           isOutput=True) for i in range(L + 1)]
        dbg_msg = nc.declare_dram_parameter("dbg_msg", [P, B, HID], dt.float32,
                                            isOutput=True)
        dbg_q = nc.declare_dram_parameter("dbg_q", [P, B, HID], dt.float32,
                                          isOutput=True)
        dbg_kv = nc.declare_dram_parameter("dbg_kv", [P, 8, H2], dt.float32,
                                           isOutput=True)
        dbg_acc = nc.declare_dram_parameter("dbg_acc", [P, 8, HEADS + HID],
                                            dt.float32, isOutput=True)
        dbg_kvg = nc.declare_dram_parameter("dbg_kvg", [P, 4, H2], dt.float32,
                                            isOutput=True)
        dbg_qd = nc.declare_dram_parameter("dbg_qd", [P, 4, HID], dt.float32,
                                           isOutput=True)
        dbg_logit = nc.declare_dram_parameter("dbg_logit", [P, 4, HEADS],
                                              dt.float32, isOutput=True)
        dbg_S = nc.declare_dram_parameter("dbg_S", [P, 4, P], dt.float32,
                                          isOutput=True)

    def dump_h(which, sbpool):
        if not debug:
            return
        for b in range(B):
            t_ = sbpool.tile([P, HID], dt.float32, tag="dbgh")
            nc.vector.tensor_copy(out=t_[:], in_=h_loc[:, b, :])
            nc.sync.dma_start(out=dbg_h[which][:, b, :], in_=t_[:])

    kv_loc = nc.dram_tensor("kv_loc", [NLp, H2], dt.float16)
    kv_tbl = nc.dram_tensor("kv_tbl", [Np, H2], dt.float16, addr_space="Shared")
    kv_pairA = nc.dram_tensor("kv_pairA", [2 * NLp, H2], dt.float16,
                              addr_space="Shared")
    kv_pairB = nc.dram_tensor("kv_pairB", [2 * NLp, H2], dt.float16,
                              addr_space="Shared")
    u_loc = nc.dram_tensor("u_loc", [NLp, HID], dt.float16)
    u_tbl = nc.dram_tensor("u_tbl", [Np, HID], dt.float16, addr_space="Shared")
    u_pair = nc.dram_tensor("u_pair", [2 * NLp, HID], dt.float16,
                            addr_space="Shared")
    S_dram = nc.dram_tensor("S_dram", [B, P, KP], dt.float16)
    St_dram = nc.dram_tensor("St_dram", [B, P, KP], dt.float16)
    barin = [nc.dram_tensor(f"barin{i}", [64, 1], dt.int32) for i in range(L + 1)]
    barout = [nc.dram_tensor(f"barout{i}", [P, 1], dt.int32)
              for i in range(L + 1)]

    GRP = 8  # z3 chunks per output DMA

    with tile.TileContext(nc) as tc:
        with tc.tile_pool(name="pers", bufs=1) as pers, \
             tc.tile_pool(name="wp", bufs=1) as wpool:
            # ---------------- persistent state ----------------
            h_loc = pers.tile([P, B, HID], dt.float32)
            hT0 = pers.tile([P, NLp], dt.float16)
            hT1 = pers.tile([HID - P, NLp], dt.float16)
            q_loc = pers.tile([P, B, HID], dt.float16)
            w_loc = pers.tile([P, B, HID], dt.float16)
            msg_loc = pers.tile([P, B, HID], dt.float16)
            acc1_loc = pers.tile([P, B, HEADS + HID], dt.float16)

            ident32 = pers.tile([P, P], dt.float32)
            make_identity(nc, ident32[:])
            ident16 = pers.tile([P, P], dt.float16)
            nc.vector.tensor_copy(out=ident16[:], in_=ident32[:])
            iota_row16 = pers.tile([P, P], dt.float16)
            nc.sync.dma_start(out=iota_row16[:], in_=c_iota_row[:, :])
            ones_row = pers.tile([1, P], dt.float16)
            nc.sync.dma_start(out=ones_row[:], in_=ones_row_p[...])

            offs_t = pers.tile([P, C], dt.int32)
            nc.sync.dma_start(out=offs_t[:], in_=off_src[:, :])
            offs2_t = pers.tile([P, C], dt.int32)
            amask_t = pers.tile([P, C], dt.float32)
            nc.sync.dma_start(out=amask_t[:], in_=amask[:, :])
            sco_t = pers.tile([P, B, 1], dt.int32)
            nc.sync.dma_start(out=sco_t[:],
                              in_=sco_p[...].rearrange("(a p) o -> p a o", p=P))
            barz_t = pers.tile([64, 1], dt.int32)
            nc.sync.dma_start(out=barz_t[:], in_=barz_p[...])

            def wtile(pp, shape, dtype, tag):
                t_ = wpool.tile(list(shape), dtype, tag=tag)
                nc.sync.dma_start(out=t_[:], in_=pp[...])
                return t_

            # ---------------- setup: h0 + S/St precompute ----------------
            with tc.tile_pool(name="sup", bufs=1) as sup, \
                 tc.tile_pool(name="supS", bufs=2) as supS, \
                 tc.tile_pool(name="psSup", bufs=2, space="PSUM") as psSup, \
                 tc.tile_pool(name="psSupB", bufs=2, space="PSUM") as psSupB:
                iotag_t = sup.tile([P, G_TBL // P, P], dt.float32)
                nc.sync.dma_start(out=iotag_t[:], in_=c_iota_col4[:, :, :])
                xspT_t = sup.tile([XS, NLp], dt.float32)
                nc.sync.dma_start(out=xspT_t[:], in_=xspT[:, :])
                gid_t = sup.tile([P, B], dt.float32)
                nc.sync.dma_start(out=gid_t[:], in_=gidcol[:, :])
                dstc_t = sup.tile([P, C], dt.float16)
                nc.sync.dma_start(out=dstc_t[:], in_=dstcol[:, :])

                Wxs_t = wtile(Wxs, [XS, HID], dt.float32, "Wxs")
                Wgp_t = wtile(Wgp, [GP, HID], dt.float32, "Wgp")
                Wep_t = wtile(Wep, [EP, HID], dt.float32, "Wep")
                gpT_t = wtile(gpT, [GP, G_TBL], dt.float32, "gpT")
                epT_t = wtile(epT, [EP, G_TBL], dt.float32, "epT")
                b_in_t = wtile(b_in, [P, HID], dt.float32, "b_in")

                # T12 [512, 192] f32
                T12 = sup.tile([P, G_TBL // P, HID], dt.float32)
                for gc in range(G_TBL // P):
                    pt = psSup.tile([P, HID], dt.float32, space="PSUM", tag="mm")
                    nc.tensor.matmul(out=pt[:], lhsT=gpT_t[:, gc * P:(gc + 1) * P],
                                     rhs=Wgp_t[:], start=True, stop=False)
                    nc.tensor.matmul(out=pt[:], lhsT=epT_t[:, gc * P:(gc + 1) * P],
                                     rhs=Wep_t[:], start=False, stop=True)
                    nc.vector.tensor_copy(out=T12[:, gc, :], in_=pt[:])

                # h0 per block
                for b in range(B):
                    gbc_ps = psSup.tile([P, P], dt.float32, space="PSUM", tag="mm")
                    nc.tensor.transpose(out=gbc_ps[:],
                                        in_=gid_t[:, b:b + 1].to_broadcast([P, P]),
                                        identity=ident32[:])
                    gbc = supS.tile([P, P], dt.float32, tag="gbc_s")
                    nc.scalar.copy(out=gbc[:], in_=gbc_ps[:])
                    hp = psSupB.tile([P, HID], dt.float32, space="PSUM", tag="acc")
                    nc.tensor.matmul(out=hp[:], lhsT=xspT_t[:, b * P:(b + 1) * P],
                                     rhs=Wxs_t[:], start=True, stop=False)
                    for gc in range(G_TBL // P):
                        og = supS.tile([P, P], dt.float32, tag="og")
                        nc.vector.tensor_tensor(out=og[:], in0=iotag_t[:, gc, :],
                                                in1=gbc[:],
                                                op=mybir.AluOpType.is_equal)
                        nc.tensor.matmul(out=hp[:], lhsT=og[:], rhs=T12[:, gc, :],
                                         start=False, stop=(gc == G_TBL // P - 1))
                    nc.vector.tensor_add(out=h_loc[:, b, :], in0=hp[:], in1=b_in_t[:])

                # S / St precompute -> DRAM
                for b in range(B):
                    S_blk = supS.tile([P, K, P], dt.float16, tag="Sblk")
                    nc.vector.tensor_tensor(
                        out=S_blk[:],
                        in0=dstc_t[:, b * K:(b + 1) * K]
                            .rearrange("p (k o) -> p k o", o=1).to_broadcast([P, K, P]),
                        in1=iota_row16[:]
                            .rearrange("p (o j) -> p o j", o=1).to_broadcast([P, K, P]),
                        op=mybir.AluOpType.is_equal)
                    nc.sync.dma_start(out=S_dram[b, :, :],
                                      in_=S_blk[:].rearrange("p k j -> p (k j)"))
                    for k0 in range(0, K, 4):
                        n4 = min(4, K - k0)
                        Stp = psSup.tile([P, 4, P], dt.float16, space="PSUM", tag="st")
                        for j in range(n4):
                            nc.tensor.transpose(out=Stp[:, j, :], in_=S_blk[:, k0 + j, :],
                                                identity=ident16[:])
                        St4 = supS.tile([P, 4, P], dt.float16, tag="St4")
                        nc.scalar.copy(out=St4[:, 0:n4, :], in_=Stp[:, 0:n4, :])
                        nc.sync.dma_start(
                            out=St_dram[b, :, k0 * P:(k0 + n4) * P],
                            in_=St4[:, 0:n4, :].rearrange("p k j -> p (k j)"))

            # (debug) h0 dump happens inside the first layer's node phase
            # ---------------- main layers ----------------
            sbN = ExitStack()
            sb = sbN.enter_context(tc.tile_pool(name="sbN", bufs=3))
            zpool = sbN.enter_context(tc.tile_pool(name="zpool", bufs=2))
            sbE = sbN.enter_context(tc.tile_pool(name="sbE", bufs=2 if _DEBUG[0] else 4))
            gat = sbN.enter_context(tc.tile_pool(name="gat", bufs=3))
            sbS = sbN.enter_context(tc.tile_pool(name="sbS", bufs=1))

            def transpose_h(ps_pool, b):
                """h_loc[:, b] -> hT0/hT1 columns (for matmul lhsT)."""
                tp = ps_pool.tile([P, 2, P], dt.float32, space="PSUM", tag="tp")
                for ci, (f0, fl) in enumerate(FC):
                    nc.tensor.transpose(out=tp[:fl, ci, :], in_=h_loc[:, b, f0:f0 + fl],
                                        identity=ident32[:])
                nc.scalar.copy(out=hT0[:, b * P:(b + 1) * P], in_=tp[:, 0, :])
                nc.scalar.copy(out=hT1[:, b * P:(b + 1) * P],
                               in_=tp[0:HID - P, 1, :])

            def layer_norm2(b0, g_t, b_t):
                """LayerNorm over blocks b0, b0+1 in one batched pass."""
                hv = h_loc[:, b0:b0 + 2, :]
                red = sb.tile([P, 2], dt.float32, tag="ln_m")
                nc.vector.tensor_reduce(out=red[:], in_=hv,
                                        axis=mybir.AxisListType.X, op=mybir.AluOpType.add)
                m = sb.tile([P, 2], dt.float32, tag="ln_mm")
                nc.vector.tensor_scalar_mul(out=m[:], in0=red[:], scalar1=1.0 / HID)
                xc = sb.tile([P, 2, HID], dt.float32, tag="ln_xc")
                nc.vector.tensor_tensor(
                    out=xc[:], in0=hv,
                    in1=m[:].rearrange("p (c o) -> p c o", o=1).to_broadcast([P, 2, HID]),
                    op=mybir.AluOpType.subtract)
                prod = sb.tile([P, 2, HID], dt.float32, tag="ln_p")
                nc.vector.tensor_tensor(out=prod[:], in0=xc[:], in1=xc[:],
                                        op=mybir.AluOpType.mult)
                sq = sb.tile([P, 2], dt.float32, tag="ln_sq")
                nc.vector.tensor_reduce(out=sq[:], in_=prod[:],
                                        axis=mybir.AxisListType.X, op=mybir.AluOpType.add)
                var = sb.tile([P, 2], dt.float32, tag="ln_v")
                nc.vector.tensor_scalar(out=var[:], in0=sq[:], scalar1=1.0 / HID,
                                        scalar2=1e-5, op0=mybir.AluOpType.mult,
                                        op1=mybir.AluOpType.add)
                rv = sb.tile([P, 2], dt.float32, tag="ln_r")
                nc.vector.reciprocal(out=rv[:], in_=var[:])
                rs = sb.tile([P, 2], dt.float32, tag="ln_rs")
                nc.scalar.sqrt(out=rs[:], in_=rv[:])
                nc.vector.tensor_tensor(
                    out=xc[:], in0=xc[:],
                    in1=rs[:].rearrange("p (c o) -> p c o", o=1).to_broadcast([P, 2, HID]),
                    op=mybir.AluOpType.mult)
                nc.vector.tensor_tensor(
                    out=xc[:], in0=xc[:],
                    in1=g_t[:].rearrange("p (o f) -> p o f", o=1).to_broadcast([P, 2, HID]),
                    op=mybir.AluOpType.mult)
                nc.vector.tensor_tensor(
                    out=h_loc[:, b0:b0 + 2, :], in0=xc[:],
                    in1=b_t[:].rearrange("p (o f) -> p o f", o=1).to_broadcast([P, 2, HID]),
                    op=mybir.AluOpType.add)

            def edge_chunk_pair(psE, psQ, psB, l_tbl, kv_pair_l, We2_t, b, kk0, nk,
                                S_t, St_t, ea_blk, phase1, kb):
                """Process chunks kk0..kk0+nk-1 (nk in {1,2}) of dst-block b."""
                t0 = b * K + kk0
                kl = kk0 - kb
                # e-projection init (PE) -> PSUM -> f16 buffer
                ep = psE.tile([P, 2, 512], dt.float32, space="PSUM", tag="ep")
                for j in range(nk):
                    nc.tensor.matmul(out=ep[:, j, 0:H2],
                                     lhsT=ea_blk[:EA, (kl + j) * P:(kl + j + 1) * P],
                                     rhs=We2_t[:], start=True, stop=True)
                kvg = gat.tile([P, 2, H2], dt.float16, tag="kvg")
                nc.scalar.copy(out=kvg[:, 0:nk, :], in_=ep[:, 0:nk, 0:H2])
                # gather-add [k|v] rows
                tbl = kv_pair_l if phase1 else l_tbl
                for j in range(nk):
                    nc.gpsimd.indirect_dma_start(
                        out=kvg[:, j, :], out_offset=None, in_=tbl[:, :],
                        in_offset=bass.IndirectOffsetOnAxis(
                            ap=(offs2_t if phase1 else offs_t)[:, t0 + j:t0 + j + 1],
                            axis=0),
                        compute_op=mybir.AluOpType.add)
                # qd expansion (PE)
                qd = psQ.tile([P, 2, HID], dt.float32, space="PSUM", tag="qd")
                for j in range(nk):
                    nc.tensor.matmul(out=qd[:, j, :],
                                     lhsT=St_t[:, kl + j, :], rhs=q_loc[:, b, :],
                                     start=True, stop=True)
                qd16 = sbE.tile([P, 2, HID], dt.float16, tag="qd16")
                nc.scalar.copy(out=qd16[:, 0:nk, :], in_=qd[:, 0:nk, :])
                # prod + logit reduce + exp + combo (DVE/ACT), 2 chunks wide
                prod = sbE.tile([P, 2, HID], dt.float16, tag="prod")
                nc.vector.tensor_tensor(out=prod[:, 0:nk, :], in0=qd16[:, 0:nk, :],
                                        in1=kvg[:, 0:nk, 0:HID],
                                        op=mybir.AluOpType.mult)
                logit = sbE.tile([P, 2, HEADS], dt.float32, tag="logit")
                nc.vector.tensor_reduce(
                    out=logit[:, 0:nk, :].rearrange("p c h -> p (c h)"),
                    in_=prod[:, 0:nk, :].rearrange("p c (h d) -> p (c h) d", h=HEADS),
                    axis=mybir.AxisListType.X, op=mybir.AluOpType.add)
                combo = sbE.tile([P, 2, HEADS + HID], dt.float16, tag="combo")
                nc.scalar.activation(out=combo[:, 0:nk, 0:HEADS], in_=logit[:, 0:nk, :],
                                     func=mybir.ActivationFunctionType.Exp)
                if any(chunk_pad[t0 + j] for j in range(nk)):
                    nc.vector.tensor_tensor(
                        out=combo[:, 0:nk, 0:HEADS],
                        in0=combo[:, 0:nk, 0:HEADS],
                        in1=amask_t[:, t0:t0 + nk]
                            .rearrange("p (c o) -> p c o", o=1).to_broadcast([P, nk, HEADS]),
                        op=mybir.AluOpType.mult)
                nc.vector.tensor_tensor(
                    out=combo[:, 0:nk, HEADS:].rearrange("p c (h d) -> p c h d", h=HEADS),
                    in0=kvg[:, 0:nk, HID:].rearrange("p c (h d) -> p c h d", h=HEADS),
                    in1=combo[:, 0:nk, 0:HEADS].rearrange("p c (h o) -> p c h o", o=1)
                        .to_broadcast([P, nk, HEADS, DH]),
                    op=mybir.AluOpType.mult)
                if (_DEBUG[0] and dbg_ctx[0] == 0 and not phase1 and b == 0
                        and kl < 4):
                    for j in range(nk):
                        tg = sbE.tile([P, H2], dt.float32, tag="dbgg")
                        nc.vector.tensor_copy(out=tg[:], in_=kvg[:, j, :])
                        nc.sync.dma_start(out=dbg_kvg[:, kl + j, :], in_=tg[:])
                        tq2 = sbE.tile([P, HID], dt.float32, tag="dbgq2")
                        nc.vector.tensor_copy(out=tq2[:], in_=qd16[:, j, :])
                        nc.sync.dma_start(out=dbg_qd[:, kl + j, :], in_=tq2[:])
                        tS = sbE.tile([P, P], dt.float32, tag="dbgS")
                        nc.vector.tensor_copy(out=tS[:], in_=S_t[:, kl + j, :])
                        nc.sync.dma_start(out=dbg_S[:, kl + j, :], in_=tS[:])
                    nc.sync.dma_start(out=dbg_logit[:, kl:kl + nk, :],
                                      in_=logit[:, 0:nk, :])
                # segment accumulate (PE)
                nkk = K1 if phase1 else K - K1
                for j in range(nk):
                    kk = kk0 + j
                    nc.tensor.matmul(out=acc[0][:], lhsT=S_t[:, kl + j, :],
                                     rhs=combo[:, j, :],
                                     start=(kk - kb == 0), stop=(kk - kb == nkk - 1),
                                     skip_group_check=True)

            # block-level edge loop helper
            acc = [None]

            def edge_block(psE, psQ, psB, l_tbl, kv_pair_l, We2_t, b, phase1):
                kb, ke = (0, K1) if phase1 else (K1, K)
                nkc = ke - kb
                S_t = sbS.tile([P, K - K1, P], dt.float16, tag="S")
                St_t = sbS.tile([P, K - K1, P], dt.float16, tag="St")
                ea_blk = sbS.tile([EA + 1, (K - K1) * P], dt.float16, tag="ea")
                nc.sync.dma_start(out=S_t[:, 0:nkc, :].rearrange("p k j -> p (k j)"),
                                  in_=S_dram[b, :, kb * P:ke * P])
                nc.sync.dma_start(out=St_t[:, 0:nkc, :].rearrange("p k j -> p (k j)"),
                                  in_=St_dram[b, :, kb * P:ke * P])
                nc.sync.dma_start(out=ea_blk[:, 0:nkc * P],
                                  in_=eaT[:, b * KP + kb * P:b * KP + ke * P])
                acc_t = psB.tile([P, HEADS + HID], dt.float32, space="PSUM", tag="acc")
                acc[0] = acc_t
                for kk0 in range(kb, ke, 2):
                    nk = min(2, ke - kk0)
                    edge_chunk_pair(psE, psQ, psB, l_tbl, kv_pair_l, We2_t,
                                    b, kk0, nk, S_t, St_t, ea_blk, phase1, kb)
                if phase1:
                    nc.vector.tensor_copy(out=acc1_loc[:, b, :], in_=acc[0][:])
                else:
                    # msg = (acc1 + acc2)[:, HEADS:] / (den + eps)
                    tot = sbE.tile([P, HEADS + HID], dt.float32, tag="tot")
                    nc.vector.tensor_tensor(out=tot[:], in0=acc[0][:],
                                            in1=acc1_loc[:, b, :],
                                            op=mybir.AluOpType.add)
                    den = sbE.tile([P, HEADS], dt.float32, tag="den")
                    nc.vector.tensor_scalar_add(out=den[:], in0=tot[:, 0:HEADS],
                                                scalar1=1e-16)
                    rden = sbE.tile([P, HEADS], dt.float32, tag="rden")
                    nc.vector.reciprocal(out=rden[:], in_=den[:])
                    nc.vector.tensor_tensor(
                        out=msg_loc[:, b, :].rearrange("p (h d) -> p h d", h=HEADS),
                        in0=tot[:, HEADS:].rearrange("p (h d) -> p h d", h=HEADS),
                        in1=rden[:].rearrange("p (h o) -> p h o", o=1)
                            .to_broadcast([P, HEADS, DH]),
                        op=mybir.AluOpType.mult)

            for l in range(L):
                Wq_t = wtile(Wq_p[l], [P, 2, HID], dt.float16, "Wq")
                WkWv_t = wtile(WkWv_p[l], [P, 2, H2], dt.float16, "WkWv")
                We2_t = wtile(We2_p[l], [EA, H2], dt.float16, "We2")
                Wo_t = wtile(Wo_p[l], [P, 2, HID], dt.float16, "Wo")
                bo_r = wtile(bo_r_p[l], [1, HID], dt.float16, "bo_r")
                bff1_r = wtile(bff1_r_p[l], [1, FFN], dt.float16, "bff1_r")
                bff2_r = wtile(bff2_r_p[l], [1, HID], dt.float16, "bff2_r")
                ln1g_t = wtile(ln1g_p[l], [P, HID], dt.float32, "ln1g")
                ln1b_t = wtile(ln1b_p[l], [P, HID], dt.float32, "ln1b")
                Wff1_t = wtile(Wff1_p[l], [P, 2, FFN], dt.float16, "Wff1")
                Wff2_t = wtile(Wff2_p[l], [P, 3, HID], dt.float16, "Wff2")
                ln2g_t = wtile(ln2g_p[l], [P, HID], dt.float32, "ln2g")
                ln2b_t = wtile(ln2b_p[l], [P, HID], dt.float32, "ln2b")

                kv_pair_l = kv_pairA if l % 2 == 0 else kv_pairB
                dbg_ctx[0] = l
                if l == 0:
                    dump_h(0, sb)

                # ---- node phase: qkv + stores ----
                with tc.tile_pool(name="psN", bufs=2, space="PSUM") as psN, \
                     tc.tile_pool(name="psNB", bufs=2, space="PSUM") as psNB:
                    for b in range(B):
                        transpose_h(psN, b)
                    for b in range(B):
                        qp = psN.tile([P, HID], dt.float32, space="PSUM", tag="mmq")
                        kvp = psNB.tile([P, H2], dt.float32, space="PSUM", tag="mmkv")
                        for ci, (f0, fl) in enumerate(FC):
                            hTt = hT0 if ci == 0 else hT1
                            lhs = hTt[:fl, b * P:(b + 1) * P]
                            nc.tensor.matmul(out=qp[:], lhsT=lhs, rhs=Wq_t[:fl, ci, :],
                                             start=(ci == 0), stop=(ci == 1))
                            nc.tensor.matmul(out=kvp[:], lhsT=lhs, rhs=WkWv_t[:fl, ci, :],
                                             start=(ci == 0), stop=(ci == 1))
                        nc.scalar.copy(out=q_loc[:, b, :], in_=qp[:])
                        if debug and l == 0:
                            tq = sb.tile([P, HID], dt.float32, tag="dbgq")
                            nc.vector.tensor_copy(out=tq[:], in_=qp[:])
                            nc.sync.dma_start(out=dbg_q[:, b, :], in_=tq[:])
                        kvf = sb.tile([P, H2], dt.float16, tag="kvf")
                        nc.scalar.copy(out=kvf[:], in_=kvp[:])
                        nc.sync.dma_start(out=kv_loc[b * P:(b + 1) * P, :], in_=kvf[:])
                        if debug and l == 0 and b < 8:
                            tkv = sb.tile([P, H2], dt.float32, tag="dbgkv")
                            nc.vector.tensor_copy(out=tkv[:], in_=kvf[:])
                            nc.sync.dma_start(out=dbg_kv[:, b, :], in_=tkv[:])
                        # scatter into pair-shared table
                        nc.gpsimd.indirect_dma_start(
                            out=kv_pair_l[0:P, :],
                            out_offset=bass.IndirectOffsetOnAxis(
                                ap=sco_t[:, b, :], axis=0),
                            in_=kvf[:], in_offset=None, oob_is_err=False)

                # ---- pair barrier: ordered after scatters via readback ----
                rb = sb.tile([64, 8], dt.float16, tag="rb")
                nc.sync.dma_start(out=rb[:], in_=kv_pair_l[0:64, 0:8])
                rbl = sb.tile([64, 1], dt.float32, tag="rbl")
                nc.vector.tensor_reduce(out=rbl[:], in_=rb[:],
                                        axis=mybir.AxisListType.X,
                                        op=mybir.AluOpType.max)
                z16 = sb.tile([64, 1], dt.int32, tag="z16")
                nc.vector.tensor_scalar(out=z16[:], in0=rbl[:], scalar1=0.0,
                                        scalar2=0.0, op0=mybir.AluOpType.mult,
                                        op1=mybir.AluOpType.mult)
                nc.vector.tensor_tensor(out=z16[:], in0=z16[:], in1=barz_t[:],
                                        op=mybir.AluOpType.add)
                nc.sync.dma_start(out=barin[l][:, :], in_=z16[:])
                nc.gpsimd.collective_compute(
                    "AllGather", mybir.AluOpType.bypass,
                    replica_groups=[[2 * i, 2 * i + 1] for i in range(n_cores // 2)],
                    ins=[barin[l][:, :]], outs=[barout[l][:, :]])
                bar_t = sb.tile([P, 1], dt.int32, tag="bar")
                nc.sync.dma_start(out=bar_t[:], in_=barout[l][:, :])
                nc.vector.tensor_tensor(out=offs2_t[:], in0=offs_t[:],
                                        in1=bar_t[:, 0:1].to_broadcast([P, C]),
                                        op=mybir.AluOpType.add)
                # delay the big collective until the pair barrier completed:
                # rewrite 8 columns of kv_loc rows with (x + 0*bar)
                kvrow = sb.tile([P, 8], dt.float16, tag="kvrow")
                nc.sync.dma_start(out=kvrow[:], in_=kv_loc[0:P, 0:8])
                bar16 = sb.tile([P, 1], dt.float16, tag="bar16")
                nc.vector.tensor_scalar_mul(out=bar16[:], in0=bar_t[:], scalar1=0.0)
                nc.vector.tensor_tensor(out=kvrow[:], in0=kvrow[:],
                                        in1=bar16[:, 0:1].to_broadcast([P, 8]),
                                        op=mybir.AluOpType.add)
                nc.sync.dma_start(out=kv_loc[0:P, 0:8], in_=kvrow[:])

                # ---- big collective (kv AllGather) ----
                nc.gpsimd.collective_compute(
                    "AllGather", mybir.AluOpType.bypass,
                    replica_groups=[list(range(n_cores))],
                    ins=[kv_loc[:, :]], outs=[kv_tbl[:, :]])

                # ---- edge loop ----
                with tc.tile_pool(name="psE", bufs=2, space="PSUM") as psE, \
                     tc.tile_pool(name="psQ", bufs=2, space="PSUM") as psQ, \
                     tc.tile_pool(name="psB", bufs=2, space="PSUM") as psB:
                    # phase 1 (pair-local) during the collective
                    for b in range(B):
                        edge_block(psE, psQ, psB, kv_tbl, kv_pair_l, We2_t, b, True)
                    # phase 2
                    for b in range(B):
                        edge_block(psE, psQ, psB, kv_tbl, kv_pair_l, We2_t, b, False)

                if debug and l == 0:
                    for b in range(B):
                        tm = sb.tile([P, HID], dt.float32, tag="dbgh")
                        nc.vector.tensor_copy(out=tm[:], in_=msg_loc[:, b, :])
                        nc.sync.dma_start(out=dbg_msg[:, b, :], in_=tm[:])
                # ---- node update: h = LN2(FFN(LN1(h + msg@Wo + bo))) ----
                with tc.tile_pool(name="psU", bufs=2, space="PSUM") as psU, \
                     tc.tile_pool(name="psUB", bufs=2, space="PSUM") as psUB:
                    for b in range(B):
                        mT = sb.tile([P, 2, P], dt.float16, tag="mT")
                        tp16m = psU.tile([P, 3, P], dt.float16, space="PSUM", tag="tpT")
                        for ci, (f0, fl) in enumerate(FC):
                            nc.tensor.transpose(out=tp16m[:fl, ci, :],
                                                in_=msg_loc[:, b, f0:f0 + fl],
                                                identity=ident16[:])
                        nc.scalar.copy(out=mT[:], in_=tp16m[:, 0:2, :])
                        yp = psUB.tile([P, HID], dt.float32, space="PSUM", tag="yf2")
                        for ci, (f0, fl) in enumerate(FC):
                            nc.tensor.matmul(out=yp[:], lhsT=mT[:fl, ci, :],
                                             rhs=Wo_t[:fl, ci, :], start=(ci == 0),
                                             stop=False)
                        nc.tensor.matmul(out=yp[:], lhsT=ones_row[:],
                                         rhs=bo_r[:], start=False, stop=True)
                        nc.vector.tensor_tensor(out=h_loc[:, b, :], in0=h_loc[:, b, :],
                                                in1=yp[:], op=mybir.AluOpType.add)
                        if b % 2 == 1:
                            layer_norm2(b - 1, ln1g_t, ln1b_t)
                    # FFN
                    for b in range(B):
                        transpose_h(psU, b)
                    for b in range(B):
                        f1p = psUB.tile([P, FFN], dt.float32, space="PSUM", tag="f1")
                        for ci, (f0, fl) in enumerate(FC):
                            hTt = hT0 if ci == 0 else hT1
                            nc.tensor.matmul(out=f1p[:], lhsT=hTt[:fl, b * P:(b + 1) * P],
                                             rhs=Wff1_t[:fl, ci, :], start=(ci == 0),
                                             stop=False)
                        nc.tensor.matmul(out=f1p[:], lhsT=ones_row[:],
                                         rhs=bff1_r[:], start=False, stop=True)
                        f1r = sb.tile([P, FFN], dt.float16, tag="f1r")
                        nc.scalar.activation(out=f1r[:], in_=f1p[:],
                                             func=mybir.ActivationFunctionType.Relu)
                        f1T = sb.tile([P, 3, P], dt.float16, tag="f1T")
                        tp16 = psU.tile([P, 3, P], dt.float16, space="PSUM", tag="tpT")
                        for ci in range(3):
                            nc.tensor.transpose(out=tp16[:, ci, :],
                                                in_=f1r[:, ci * P:(ci + 1) * P],
                                                identity=ident16[:])
                        nc.scalar.copy(out=f1T[:], in_=tp16[:])
                        f2p = psUB.tile([P, HID], dt.float32, space="PSUM", tag="yf2")
                        for ci in range(3):
                            nc.tensor.matmul(out=f2p[:], lhsT=f1T[:, ci, :],
                                             rhs=Wff2_t[:, ci, :], start=(ci == 0),
                                             stop=False)
                        nc.tensor.matmul(out=f2p[:], lhsT=ones_row[:],
                                         rhs=bff2_r[:], start=False, stop=True)
                        nc.vector.tensor_tensor(out=h_loc[:, b, :], in0=h_loc[:, b, :],
                                                in1=f2p[:], op=mybir.AluOpType.add)
                        if b % 2 == 1:
                            layer_norm2(b - 1, ln2g_t, ln2b_t)
                if True:
                    dump_h(l + 1, sb)

            # ---------------- edge head ----------------
            W1a_t = wtile(W1a, [P, 2, HID], dt.float16, "W1a")
            W1b_t = wtile(W1b, [P, 2, HID], dt.float16, "W1b")
            W1c_t = wtile(W1c, [EA + 1, HID], dt.float16, "W1c")
            W2_t = wtile(W2, [P, 2, Z2], dt.float16, "W2")
            W3_t = wtile(W3, [Z2, 1], dt.float16, "W3")

            with tc.tile_pool(name="psH", bufs=2, space="PSUM") as psH, \
                 tc.tile_pool(name="psHB", bufs=2, space="PSUM") as psHB:
                for b in range(B):
                    transpose_h(psH, b)
                for b in range(B):
                    up = psH.tile([P, HID], dt.float32, space="PSUM", tag="mmu")
                    wp_ = psHB.tile([P, HID], dt.float32, space="PSUM", tag="mmw")
                    for ci, (f0, fl) in enumerate(FC):
                        hTt = hT0 if ci == 0 else hT1
                        lhs = hTt[:fl, b * P:(b + 1) * P]
                        nc.tensor.matmul(out=up[:], lhsT=lhs, rhs=W1a_t[:fl, ci, :],
                                         start=(ci == 0), stop=(ci == 1))
                        nc.tensor.matmul(out=wp_[:], lhsT=lhs, rhs=W1b_t[:fl, ci, :],
                                         start=(ci == 0), stop=(ci == 1))
                    uf16 = sb.tile([P, HID], dt.float16, tag="uf16")
                    nc.scalar.copy(out=uf16[:], in_=up[:])
                    nc.sync.dma_start(out=u_loc[b * P:(b + 1) * P, :], in_=uf16[:])
                    nc.gpsimd.indirect_dma_start(
                        out=u_pair[0:P, :],
                        out_offset=bass.IndirectOffsetOnAxis(ap=sco_t[:, b, :], axis=0),
                        in_=uf16[:], in_offset=None, oob_is_err=False)
                    nc.scalar.copy(out=w_loc[:, b, :], in_=wp_[:])

            # pair barrier for u + big u collective
            rb = sb.tile([64, 8], dt.float16, tag="rb")
            nc.sync.dma_start(out=rb[:], in_=u_pair[0:64, 0:8])
            rbl = sb.tile([64, 1], dt.float32, tag="rbl")
            nc.vector.tensor_reduce(out=rbl[:], in_=rb[:], axis=mybir.AxisListType.X,
                                    op=mybir.AluOpType.max)
            z16 = sb.tile([64, 1], dt.int32, tag="z16")
            nc.vector.tensor_scalar(out=z16[:], in0=rbl[:], scalar1=0.0, scalar2=0.0,
                                    op0=mybir.AluOpType.mult, op1=mybir.AluOpType.mult)
            nc.vector.tensor_tensor(out=z16[:], in0=z16[:], in1=barz_t[:],
                                    op=mybir.AluOpType.add)
            nc.sync.dma_start(out=barin[L][:, :], in_=z16[:])
            nc.gpsimd.collective_compute(
                "AllGather", mybir.AluOpType.bypass,
                replica_groups=[[2 * i, 2 * i + 1] for i in range(n_cores // 2)],
                ins=[barin[L][:, :]], outs=[barout[L][:, :]])
            bar_t = sb.tile([P, 1], dt.int32, tag="bar")
            nc.sync.dma_start(out=bar_t[:], in_=barout[L][:, :])
            nc.vector.tensor_tensor(out=offs2_t[:], in0=offs_t[:],
                                    in1=bar_t[:, 0:1].to_broadcast([P, C]),
                                    op=mybir.AluOpType.add)
            kvrow = sb.tile([P, 8], dt.float16, tag="kvrow")
            nc.sync.dma_start(out=kvrow[:], in_=u_loc[0:P, 0:8])
            bar16 = sb.tile([P, 1], dt.float16, tag="bar16")
            nc.vector.tensor_scalar_mul(out=bar16[:], in0=bar_t[:], scalar1=0.0)
            nc.vector.tensor_tensor(out=kvrow[:], in0=kvrow[:],
                                    in1=bar16[:, 0:1].to_broadcast([P, 8]),
                                    op=mybir.AluOpType.add)
            nc.sync.dma_start(out=u_loc[0:P, 0:8], in_=kvrow[:])
            nc.gpsimd.collective_compute(
                "AllGather", mybir.AluOpType.bypass,
                replica_groups=[list(range(n_cores))],
                ins=[u_loc[:, :]], outs=[u_tbl[:, :]])

            def head_chunk_pair(psE, psQ, psB, psZ, b, kk0, nk, St_t, ea_blk,
                                phase1, zbuf, kb):
                t0 = b * K + kk0
                kl = kk0 - kb
                z1p = psE.tile([P, 2, HID], dt.float32, space="PSUM", tag="z1")
                for j in range(nk):
                    nc.tensor.matmul(out=z1p[:, j, :], lhsT=St_t[:, kl + j, :],
                                     rhs=w_loc[:, b, :], start=True, stop=False)
                    nc.tensor.matmul(out=z1p[:, j, :],
                                     lhsT=ea_blk[:, (kl + j) * P:(kl + j + 1) * P],
                                     rhs=W1c_t[:], start=False, stop=True)
                z1g = gat.tile([P, 2, HID], dt.float16, tag="z1g")
                nc.vector.tensor_copy(out=z1g[:, 0:nk, :], in_=z1p[:, 0:nk, :])
                tbl = u_pair if phase1 else u_tbl
                for j in range(nk):
                    nc.gpsimd.indirect_dma_start(
                        out=z1g[:, j, :], out_offset=None, in_=tbl[:, :],
                        in_offset=bass.IndirectOffsetOnAxis(
                            ap=(offs2_t if phase1 else offs_t)[:, t0 + j:t0 + j + 1],
                            axis=0),
                        compute_op=mybir.AluOpType.add)
                z1r = sbE.tile([P, 2, HID], dt.float16, tag="z1r")
                nc.scalar.activation(out=z1r[:, 0:nk, :], in_=z1g[:, 0:nk, :],
                                     func=mybir.ActivationFunctionType.Relu)
                z1T = sbE.tile([P, 4, P], dt.float16, tag="z1T")
                tps = psQ.tile([P, 4, P], dt.float16, space="PSUM", tag="z1T")
                for j in range(nk):
                    for ci, (f0, fl) in enumerate(FC):
                        nc.tensor.transpose(out=tps[:fl, 2 * j + ci, :],
                                            in_=z1r[:, j, f0:f0 + fl],
                                            identity=ident16[:])
                nc.vector.tensor_copy(out=z1T[:], in_=tps[:])
                z2p = psB.tile([Z2, 2, P], dt.float32, space="PSUM", tag="z2")
                for j in range(nk):
                    for ci, (f0, fl) in enumerate(FC):
                        nc.tensor.matmul(out=z2p[:, j, :], lhsT=W2_t[:fl, ci, :],
                                         rhs=z1T[:fl, 2 * j + ci, :],
                                         start=(ci == 0), stop=(ci == 1))
                z2r = sbE.tile([Z2, 2, P], dt.float16, tag="z2r")
                nc.scalar.activation(out=z2r[:, 0:nk, :], in_=z2p[:, 0:nk, :],
                                     func=mybir.ActivationFunctionType.Relu)
                z3ps = psZ.tile([1, 2, P], dt.float32, space="PSUM", tag="z3")
                for j in range(nk):
                    nc.tensor.matmul(out=z3ps[:, j, :], lhsT=W3_t[:, :],
                                     rhs=z2r[:, j, :], start=True, stop=True)
                nc.scalar.activation(
                    out=zbuf[:, kl * P:(kl + nk) * P],
                    in_=z3ps[:, 0:nk, :].rearrange("o c j -> o (c j)"),
                    func=mybir.ActivationFunctionType.Copy,
                    bias=float(b_e3_const))

            with tc.tile_pool(name="psE2", bufs=2, space="PSUM") as psE2, \
                 tc.tile_pool(name="psQ2", bufs=2, space="PSUM") as psQ2, \
                 tc.tile_pool(name="psB2", bufs=2, space="PSUM") as psB2, \
                 tc.tile_pool(name="psZ", bufs=2, space="PSUM") as psZ:
                for phase1 in (True, False):
                    kb, ke = (0, K1) if phase1 else (K1, K)
                    nkc = ke - kb
                    for b in range(B):
                        St_t = sbS.tile([P, K - K1, P], dt.float16, tag="St")
                        ea_blk = sbS.tile([EA + 1, (K - K1) * P], dt.float16, tag="ea")
                        nc.sync.dma_start(
                            out=St_t[:, 0:nkc, :].rearrange("p k j -> p (k j)"),
                            in_=St_dram[b, :, kb * P:ke * P])
                        nc.sync.dma_start(
                            out=ea_blk[:, 0:nkc * P],
                            in_=eaT[:, b * KP + kb * P:b * KP + ke * P])
                        zbuf = zpool.tile([1, (K - K1) * P], dt.float32, tag="zb")
                        for kk0 in range(kb, ke, 2):
                            nk = min(2, ke - kk0)
                            head_chunk_pair(psE2, psQ2, psB2, psZ, b, kk0, nk,
                                            St_t, ea_blk, phase1, zbuf, kb)
                        nc.sync.dma_start(
                            out=out_z[:, (b * K + kb) * P:(b * K + ke) * P],
                            in_=zbuf[:, 0:nkc * P])
            sbN.close()

    return nc


# ----------------------------------------------------------------------------
# public entry
# ----------------------------------------------------------------------------

def _run(inputs, n_cores, runner):
    shards, meta = _host_prep(
        inputs["x"], inputs["edge_index"], inputs["edge_attr"], inputs["batch"],
        inputs["group_ptr"], inputs["time_group_ids"], inputs["group_probs"],
        inputs["splitter_probs"], inputs["endpoint_preds"], n_cores)
    w, b_e3c = _host_weights(
        inputs["group_probs"], inputs["endpoint_preds"],
        *[inputs[k] for k in [
            "W_in", "b_in", "Wq", "Wk", "Wv", "We", "Wo", "bo",
            "ln1_g", "ln1_b", "W_ff1", "b_ff1", "W_ff2", "b_ff2",
            "ln2_g", "ln2_b", "W_e1", "b_e1", "W_e2", "b_e2", "W_e3", "b_e3"]])
    nc = build_program(meta, b_e3c, n_cores)
    in_maps = []
    for c in range(n_cores):
        m = dict(shards[c])
        m.update(w)
        in_maps.append(m)
    results = runner(nc, in_maps)
    E = meta["E"]
    out = np.zeros((E, 1), np.float32)
    for c in range(n_cores):
        z = np.asarray(results[c]["out_z"]).reshape(-1)
        eid = meta["eid_sh"][c]
        valid = eid >= 0
        out[eid[valid], 0] = z[valid]
    return out


def kernel(**inputs):
    from concourse.bass_utils import run_bass_kernel_spmd

    n_cores = 8

    def runner(nc, in_maps):
        split_excess_waits(nc, max_waits=1)
        br = run_bass_kernel_spmd(nc, in_maps, core_ids=list(range(n_cores)))
        return br.results

    return _run(inputs, n_cores, runner)
